# revision 1
# baseline (speedup 1.0000x reference)
"""Trainium2 Bass kernel for nn_ConvBlock (Chebyshev graph conv + BatchNorm + ReLU).

Sharding: data-parallel over batch (B=8 -> 1 sample per NeuronCore).
Per core: Chebyshev recursion via indirect-DMA row gathers + PE scatter-matmuls
(edge weights folded into host-built one-hot scatter blocks), K-stacked GEMM with
host-folded power-basis weights, BatchNorm stats on device (combined across cores
on host between two launches), normalize+ReLU+transpose on device.
"""
import os, sys
sys.path.insert(0, '/opt/trn_rl_repo')
import numpy as np
from contextlib import ExitStack

import concourse.bass as bass
import concourse.tile as tile
from concourse import bacc, mybir
from concourse.bass_utils import run_bass_kernel_spmd
from concourse.masks import make_identity

B, V, E = 8, 12288, 98304
FIN, FOUT, K = 256, 256, 4
EPS = 1e-5
P = 128
GSZ = 64            # dst-group node window (S_w block width)
NVT = V // P        # 96 vtiles (group pairs)
NCH = 24            # GEMM chunks of 512 nodes
CHV = NVT // NCH    # 4 vtiles per chunk

F32 = mybir.dt.float32
AF = mybir.ActivationFunctionType

_cache = {}


def _build_schedule(edge_src, edge_dst, edge_weight):
    """Group edges by 64-node dst windows, pad each group to multiples of 128."""
    g_of_e = edge_dst // GSZ
    order = np.argsort(g_of_e, kind='stable')
    NG = V // GSZ
    counts = np.bincount(g_of_e, minlength=NG)
    sub_of_g = np.maximum(1, (counts + P - 1) // P)   # subtiles per group
    ST = int(sub_of_g.sum())
    idx_np = np.zeros((ST, P), np.int32)              # src per (subtile, lane)
    sw = np.zeros((ST, P, GSZ), np.float32)           # scatter blocks
    vt_subs = [[] for _ in range(NVT)]                # subtile ids per vtile half
    t = 0
    pos = 0
    for g in range(NG):
        eg = order[pos:pos + counts[g]]
        pos += counts[g]
        for s in range(sub_of_g[g]):
            part = eg[s * P:(s + 1) * P]
            n = len(part)
            idx_np[t, :n] = edge_src[part]
            sw[t, np.arange(n), edge_dst[part] - g * GSZ] = edge_weight[part]
            vt_subs[g // 2].append((t, g % 2))
            t += 1
    assert t == ST
    return idx_np, sw, vt_subs, ST


def _fold_weights(weight):
    # out = sum_k T_k(L) x W_k ; T0=I, T1=L, T2=2L^2-1, T3=4L^3-3L
    # power basis z_j = L^j x :  out = sum_j z_j Wf_j
    W = weight
    Wf = np.stack([W[0] - W[2], W[1] - 3.0 * W[3], 2.0 * W[2], 4.0 * W[3]])
    # [(j,i), o] -> tiles [8, 128, 256]
    return Wf.reshape(K * FIN, FOUT).reshape(8, P, FOUT).copy()


def _build_launch_a(ST, vt_subs):
    nc = bacc.Bacc("TRN2", target_bir_lowering=False, debug=False, num_devices=8)
    xb = nc.dram_tensor("xb", [V, FIN], F32, kind="ExternalInput").ap()
    msg0 = nc.dram_tensor("msg0", [P, ST, FIN], F32, kind="ExternalInput").ap()
    idx = nc.dram_tensor("idx", [P, ST], mybir.dt.int32, kind="ExternalInput").ap()
    swt = nc.dram_tensor("swt", [P, ST * GSZ], F32, kind="ExternalInput").ap()
    wf = nc.dram_tensor("wf", [8, P, FOUT], F32, kind="ExternalInput").ap()
    rawT = nc.dram_tensor("rawT", [2, P, V], F32, kind="ExternalOutput").ap()
    stats = nc.dram_tensor("stats", [P, 4], F32, kind="ExternalOutput").ap()
    zd = [xb] + [nc.dram_tensor(f"z{j}", [V, FIN], F32).ap() for j in (1, 2, 3)]

    with tile.TileContext(nc) as tc, ExitStack() as ctx:
        cpool = ctx.enter_context(tc.tile_pool(name="const", bufs=1))
        idx_t = cpool.tile([P, ST], mybir.dt.int32, tag="idx")
        nc.sync.dma_start(idx_t[:], idx[:, :])
        ident = cpool.tile([P, P], F32, tag="id")
        make_identity(nc, ident[:])
        wf_t = cpool.tile([P, 8 * FOUT], F32, tag="wf")
        nc.sync.dma_start(wf_t[:].rearrange("p (k o) -> p k o", k=8), wf.transpose([1, 0, 2]))

        # ---- Chebyshev (power-basis) recursion: z_j = L z_{j-1} ----
        with ExitStack() as rctx:
            swp = rctx.enter_context(tc.tile_pool(name="swp", bufs=3))
            msgp = rctx.enter_context(tc.tile_pool(name="msgp", bufs=3))
            outp = rctx.enter_context(tc.tile_pool(name="outp", bufs=3))
            psp = rctx.enter_context(tc.tile_pool(name="psp", bufs=4, space="PSUM"))
            for j in (1, 2, 3):
                for vt in range(NVT):
                    subs = vt_subs[vt]
                    nst = len(subs)
                    t0 = subs[0][0]
                    sw_t = swp.tile([P, nst * GSZ], F32, tag="sw")
                    nc.sync.dma_start(sw_t[:], swt[:, t0 * GSZ:(t0 + nst) * GSZ])
                    msg_t = msgp.tile([P, nst * FIN], F32, tag="msg")
                    if j == 1:
                        nc.sync.dma_start(
                            msg_t[:].rearrange("p (t f) -> p t f", t=nst),
                            msg0[:, t0:t0 + nst, :])
                    else:
                        for s in range(nst):
                            nc.gpsimd.indirect_dma_start(
                                out=msg_t[:, s * FIN:(s + 1) * FIN], out_offset=None,
                                in_=zd[j - 1][:, :],
                                in_offset=bass.IndirectOffsetOnAxis(
                                    ap=idx_t[:, t0 + s:t0 + s + 1], axis=0))
                    ps = psp.tile([P, FIN], F32, tag="acc")
                    half_count = [sum(1 for _, h in subs if h == hh) for hh in (0, 1)]
                    seen = [0, 0]
                    for s, (t, h) in enumerate(subs):
                        nc.tensor.matmul(
                            ps[h * GSZ:(h + 1) * GSZ, :],
                            sw_t[:, s * GSZ:(s + 1) * GSZ],
                            msg_t[:, s * FIN:(s + 1) * FIN],
                            start=(seen[h] == 0), stop=(seen[h] == half_count[h] - 1))
                        seen[h] += 1
                    o_t = outp.tile([P, FIN], F32, tag="zo")
                    nc.scalar.activation(o_t[:], ps[:], AF.Copy)
                    nc.sync.dma_start(
                        zd[j].rearrange("(vt p) f -> vt p f", p=P)[vt], o_t[:])

        # ---- GEMM + BN stats ----
        with ExitStack() as gctx:
            zin = gctx.enter_context(tc.tile_pool(name="zin", bufs=2))
            ztp = gctx.enter_context(tc.tile_pool(name="ztp", bufs=2))
            big = gctx.enter_context(tc.tile_pool(name="big", bufs=1))
            psT = gctx.enter_context(tc.tile_pool(name="psT", bufs=4, space="PSUM"))
            psG = gctx.enter_context(tc.tile_pool(name="psG", bufs=2, space="PSUM"))
            rawT_sb = big.tile([P, 2 * V], F32, tag="rawT")
            stats_sb = big.tile([P, 2 * NCH * 6], F32, tag="stats")
            for c in range(NCH):
                zin_t = zin.tile([P, K * CHV * FIN], F32, tag="zin")
                for j in range(K):
                    nc.sync.dma_start(
                        zin_t[:, j * CHV * FIN:(j + 1) * CHV * FIN]
                        .rearrange("p (vt f) -> p vt f", vt=CHV),
                        zd[j].rearrange("(c vt p) f -> c p vt f", vt=CHV, p=P)[c])
                zT = ztp.tile([P, 8 * 512], F32, tag="zT")
                for j in range(K):
                    for vt in range(CHV):
                        for fh in range(2):
                            pt = psT.tile([P, P], F32, tag="pt")
                            nc.tensor.transpose(
                                pt[:],
                                zin_t[:, (j * CHV + vt) * FIN + fh * P:
                                      (j * CHV + vt) * FIN + fh * P + P],
                                ident[:])
                            kt = j * 2 + fh
                            eng = nc.vector if (vt + fh) % 2 == 0 else nc.scalar
                            if eng is nc.vector:
                                nc.vector.tensor_copy(zT[:, kt * 512 + vt * P: kt * 512 + vt * P + P], pt[:])
                            else:
                                nc.scalar.activation(zT[:, kt * 512 + vt * P: kt * 512 + vt * P + P], pt[:], AF.Copy)
                for oh in range(2):
                    pg = psG.tile([P, 512], F32, tag="pg")
                    for kt in range(8):
                        nc.tensor.matmul(
                            pg[:], wf_t[:, kt * FOUT + oh * P: kt * FOUT + oh * P + P],
                            zT[:, kt * 512:(kt + 1) * 512],
                            start=(kt == 0), stop=(kt == 7))
                    nc.vector.bn_stats(stats_sb[:, (oh * NCH + c) * 6:(oh * NCH + c) * 6 + 6], pg[:])
                    nc.scalar.activation(rawT_sb[:, oh * V + c * 512: oh * V + (c + 1) * 512], pg[:], AF.Copy)
            for oh in range(2):
                nc.sync.dma_start(rawT[oh], rawT_sb[:, oh * V:(oh + 1) * V])
            aggr = big.tile([P, 4], F32, tag="aggr")
            for oh in range(2):
                nc.vector.bn_aggr(aggr[:, oh * 2:oh * 2 + 2],
                                  stats_sb[:, oh * NCH * 6:(oh + 1) * NCH * 6])
            # stats out: [mean_h0, ex2_h0, mean_h1, ex2_h1]
            so = big.tile([P, 4], F32, tag="so")
            for oh in range(2):
                m = aggr[:, oh * 2:oh * 2 + 1]
                v_ = aggr[:, oh * 2 + 1:oh * 2 + 2]
                nc.vector.tensor_copy(so[:, oh * 2:oh * 2 + 1], m)
                nc.vector.tensor_tensor(out=so[:, oh * 2 + 1:oh * 2 + 2], in0=m, in1=m,
                                        op=mybir.AluOpType.mult)
                nc.vector.tensor_tensor(out=so[:, oh * 2 + 1:oh * 2 + 2],
                                        in0=so[:, oh * 2 + 1:oh * 2 + 2], in1=v_,
                                        op=mybir.AluOpType.add)
            nc.sync.dma_start(stats[:, :], so[:])
    nc.compile()
    return nc


def _build_launch_b():
    nc = bacc.Bacc("TRN2", target_bir_lowering=False, debug=False, num_devices=8)
    rawT = nc.dram_tensor("rawT", [2, P, V], F32, kind="ExternalInput").ap()
    sc = nc.dram_tensor("sc", [P, 2], F32, kind="ExternalInput").ap()
    sh = nc.dram_tensor("sh", [P, 2], F32, kind="ExternalInput").ap()
    out = nc.dram_tensor("out", [V, FOUT], F32, kind="ExternalOutput").ap()
    CH2 = 8           # vtiles per chunk
    NC2 = NVT // CH2  # 12 chunks
    with tile.TileContext(nc) as tc, ExitStack() as ctx:
        cpool = ctx.enter_context(tc.tile_pool(name="const", bufs=1))
        ident = cpool.tile([P, P], F32, tag="id")
        make_identity(nc, ident[:])
        sc_t = cpool.tile([P, 2], F32, tag="sc")
        sh_t = cpool.tile([P, 2], F32, tag="sh")
        nc.sync.dma_start(sc_t[:], sc[:, :])
        nc.sync.dma_start(sh_t[:], sh[:, :])
        pool = ctx.enter_context(tc.tile_pool(name="sb", bufs=2))
        psp = ctx.enter_context(tc.tile_pool(name="ps", bufs=4, space="PSUM"))
        for c in range(NC2):
            nt = pool.tile([P, 2 * CH2 * P], F32, tag="nt")
            for oh in range(2):
                nc.sync.dma_start(nt[:, oh * CH2 * P:(oh + 1) * CH2 * P],
                                  rawT[oh][:, c * CH2 * P:(c + 1) * CH2 * P])
            for oh in range(2):
                nc.scalar.activation(
                    nt[:, oh * CH2 * P:(oh + 1) * CH2 * P],
                    nt[:, oh * CH2 * P:(oh + 1) * CH2 * P],
                    AF.Relu, bias=sh_t[:, oh:oh + 1], scale=sc_t[:, oh:oh + 1])
            ot = pool.tile([P, CH2 * FOUT], F32, tag="ot")
            for vt in range(CH2):
                pt = psp.tile([P, FOUT], F32, tag="pt")
                for oh in range(2):
                    nc.tensor.transpose(
                        pt[:, oh * P:(oh + 1) * P],
                        nt[:, oh * CH2 * P + vt * P: oh * CH2 * P + (vt + 1) * P],
                        ident[:])
                eng = vt % 2
                if eng == 0:
                    nc.vector.tensor_copy(ot[:, vt * FOUT:(vt + 1) * FOUT], pt[:])
                else:
                    nc.scalar.activation(ot[:, vt * FOUT:(vt + 1) * FOUT], pt[:], AF.Copy)
            nc.sync.dma_start(
                out.rearrange("(c vt p) f -> c p vt f", vt=CH2, p=P)[c],
                ot[:].rearrange("p (vt f) -> p vt f", vt=CH2))
    nc.compile()
    return nc


def kernel(x, edge_weight, weight, bias, gamma, beta, edge_src, edge_dst):
    x = np.asarray(x, np.float32)
    edge_weight = np.asarray(edge_weight, np.float32)
    weight = np.asarray(weight, np.float32)
    gamma = np.asarray(gamma, np.float32)
    beta = np.asarray(beta, np.float32)
    edge_src = np.asarray(edge_src, np.int32)
    edge_dst = np.asarray(edge_dst, np.int32)

    idx_np, sw, vt_subs, ST = _build_schedule(edge_src, edge_dst, edge_weight)
    key = ("A", ST, tuple(len(s) for s in vt_subs))
    if key not in _cache:
        _cache[key] = _build_launch_a(ST, vt_subs)
    ncA = _cache[key]
    if "B" not in _cache:
        _cache["B"] = _build_launch_b()
    ncB = _cache["B"]

    wf = _fold_weights(weight)
    swt = np.ascontiguousarray(sw.transpose(1, 0, 2)).reshape(P, ST * GSZ)
    idx_t = np.ascontiguousarray(idx_np.T)             # [P, ST]
    in_maps = []
    for b in range(B):
        msg0 = x[b][idx_np.reshape(-1)].reshape(ST, P, FIN).transpose(1, 0, 2)
        in_maps.append({
            "xb": np.ascontiguousarray(x[b]),
            "msg0": np.ascontiguousarray(msg0),
            "idx": idx_t, "swt": swt, "wf": wf,
        })
    resA = run_bass_kernel_spmd(ncA, in_maps, core_ids=list(range(B)))

    # host: combine BN stats across cores (equal counts -> simple average)
    st = np.stack([resA.results[b]["stats"] for b in range(B)])   # [B, 128, 4]
    mean = st[:, :, [0, 2]].mean(0)                               # [128, 2]
    ex2 = st[:, :, [1, 3]].mean(0)
    var = ex2 - mean * mean
    g2 = gamma.reshape(2, P).T                                    # [128, 2]
    b2 = beta.reshape(2, P).T
    scale = (g2 / np.sqrt(var + EPS)).astype(np.float32)
    shift = (b2 - mean * scale).astype(np.float32)

    in_maps_b = [{"rawT": resA.results[b]["rawT"], "sc": scale, "sh": shift}
                 for b in range(B)]
    resB = run_bass_kernel_spmd(ncB, in_maps_b, core_ids=list(range(B)))
    global _last_inmaps
    _last_inmaps = {key: in_maps, "B": in_maps_b}
    out = np.stack([resB.results[b]["out"] for b in range(B)])
    # bias cancels inside training-mode BN (shifts the mean only); gamma/beta applied above
    return out.astype(np.float32)



# revision 12
# speedup vs baseline: 3.1483x; 3.1483x over previous
"""Trainium2 Bass kernel for nn_ConvBlock (Chebyshev graph conv + BatchNorm + ReLU).

Sharding: data-parallel over batch (B=8 -> 1 sample per NeuronCore).
Per core: Chebyshev recursion in the power basis (z_j = L^j x) with edge-row
gathers done by batched SWDGE dma_gather (bf16, one call per 8 node-vtiles)
and scatter via one-hot matmuls (edge weights folded into host-built scatter
blocks). The recursion also emits z_j pre-transposed (feature-major zT) so the
K-stacked GEMM consumes it directly with no on-device transposes. BN stats are
computed on device, combined across cores on host between two launches;
launch B applies scale/shift + ReLU and transposes back to node-major.
All matmul/DMA traffic is bf16 (PSUM accumulation fp32); host does fp32<->bf16.
"""
import os, sys
sys.path.insert(0, '/opt/trn_rl_repo')
import numpy as np
import ml_dtypes
from contextlib import ExitStack

import concourse.bass as bass
import concourse.tile as tile
from concourse import bacc, mybir
from concourse.bass_utils import run_bass_kernel_spmd
from concourse.masks import make_identity
from concourse.library_config import mlp

B, V, E = 8, 12288, 98304
FIN, FOUT, K = 256, 256, 4
EPS = 1e-5
P = 128
GSZ = 64            # dst-group node window (S_w block width)
NVT = V // P        # 96 vtiles (group pairs)
G = 8               # vtiles per gather/compute group in the recursion
NGRP = NVT // G     # 12 groups
NCH = 24            # GEMM chunks of 512 nodes
CW = V // NCH       # 512

F32 = mybir.dt.float32
BF16 = mybir.dt.bfloat16
I16 = mybir.dt.int16
AF = mybir.ActivationFunctionType
BF16_NP = ml_dtypes.bfloat16

_cache = {}


def _build_schedule(edge_src, edge_dst, edge_weight):
    """Group edges by 64-node dst windows, pad each group to multiples of 128."""
    g_of_e = edge_dst // GSZ
    order = np.argsort(g_of_e, kind='stable')
    NG = V // GSZ
    counts = np.bincount(g_of_e, minlength=NG)
    sub_of_g = np.maximum(1, (counts + P - 1) // P)   # subtiles per group
    ST = int(sub_of_g.sum())
    idx_np = np.zeros((ST, P), np.int32)              # src per (subtile, lane)
    sw = np.zeros((ST, P, GSZ), np.float32)           # scatter blocks
    vt_subs = [[] for _ in range(NVT)]                # subtile ids per vtile half
    t = 0
    pos = 0
    for g in range(NG):
        eg = order[pos:pos + counts[g]]
        pos += counts[g]
        for s in range(sub_of_g[g]):
            part = eg[s * P:(s + 1) * P]
            n = len(part)
            idx_np[t, :n] = edge_src[part]
            sw[t, np.arange(n), edge_dst[part] - g * GSZ] = edge_weight[part]
            vt_subs[g // 2].append((t, g % 2))
            t += 1
    assert t == ST
    return idx_np, sw, vt_subs, ST


def _fold_weights(weight):
    # out = sum_k T_k(L) x W_k ; T0=I, T1=L, T2=2L^2-1, T3=4L^3-3L
    # power basis z_j = L^j x :  out = sum_j z_j Wf_j
    W = weight
    Wf = np.stack([W[0] - W[2], W[1] - 3.0 * W[3], 2.0 * W[2], 4.0 * W[3]])
    # [(j,i), o] -> tiles [8, 128, 256]
    return Wf.reshape(K * FIN, FOUT).reshape(8, P, FOUT).astype(BF16_NP)


def _wrap_idx(idx_np, ST):
    """dma_gather index layout: idx i at [i%16, i//16], replicated to 128 rows."""
    # idx_np: [ST, 128] int; per subtile t lane l -> position t*128+l
    blk = idx_np.astype(np.int16).reshape(ST, 8, 16).transpose(2, 0, 1).reshape(16, ST * 8)
    return np.ascontiguousarray(np.tile(blk, (8, 1)))  # [128, ST*8]


def _build_launch_a(ST, vt_subs):
    nc = bacc.Bacc("TRN2", target_bir_lowering=False, debug=False, num_devices=8)
    xb = nc.dram_tensor("xb", [V, FIN], BF16, kind="ExternalInput").ap()
    msg0 = nc.dram_tensor("msg0", [P, ST, FIN], BF16, kind="ExternalInput").ap()
    idx = nc.dram_tensor("idx", [P, ST * 8], I16, kind="ExternalInput").ap()
    swt = nc.dram_tensor("swt", [P, ST * GSZ], BF16, kind="ExternalInput").ap()
    wf = nc.dram_tensor("wf", [8, P, FOUT], BF16, kind="ExternalInput").ap()
    rawT = nc.dram_tensor("rawT", [2, P, V], BF16, kind="ExternalOutput").ap()
    stats = nc.dram_tensor("stats", [P, 4], F32, kind="ExternalOutput").ap()
    zn = [xb] + [nc.dram_tensor(f"z{j}", [V, FIN], BF16).ap() for j in (1, 2, 3)]

    # group boundaries: (t0, nst, [(vt, subs)...]) per G-vtile group
    groups = []
    for g in range(NGRP):
        vts = list(range(g * G, (g + 1) * G))
        subs_all = [s for vt in vts for s in vt_subs[vt]]
        groups.append((subs_all[0][0], len(subs_all), vts))

    with tile.TileContext(nc) as tc, ExitStack() as ctx:
        cpool = ctx.enter_context(tc.tile_pool(name="const", bufs=1))
        nc.gpsimd.load_library(mlp)
        idx_t = cpool.tile([P, ST * 8], I16, tag="idx")
        nc.sync.dma_start(idx_t[:], idx[:, :])
        wf_t = cpool.tile([P, 8 * FOUT], BF16, tag="wf")
        nc.sync.dma_start(wf_t[:].rearrange("p (k o) -> p k o", k=8), wf.transpose([1, 0, 2]))

        # ---- Chebyshev (power-basis) recursion: z_j = L z_{j-1} ----
        with ExitStack() as rctx:
            swp = rctx.enter_context(tc.tile_pool(name="swp", bufs=2))
            msgp = rctx.enter_context(tc.tile_pool(name="msgp", bufs=2))
            outp = rctx.enter_context(tc.tile_pool(name="outp", bufs=2))
            psp = rctx.enter_context(tc.tile_pool(name="psp", bufs=4, space="PSUM"))
            for j in (1, 2, 3):
                src = zn[j - 1]
                for (t0, nst, vts) in groups:
                    g = vts[0] // G
                    sw_t = swp.tile([P, nst * GSZ], BF16, tag="sw")
                    nc.sync.dma_start(sw_t[:], swt[:, t0 * GSZ:(t0 + nst) * GSZ])
                    msg_t = msgp.tile([P, nst * FIN], BF16, tag="msg")
                    if j == 1:
                        nc.sync.dma_start(
                            msg_t[:].rearrange("p (t f) -> p t f", t=nst),
                            msg0[:, t0:t0 + nst, :])
                    else:
                        # SWDGE ucode caps one gather call at 1024 descriptors
                        for s0 in range(0, nst, 8):
                            n = min(8, nst - s0)
                            nc.gpsimd.dma_gather(
                                msg_t[:].rearrange("p (t f) -> p t f", t=nst)[:, s0:s0 + n, :],
                                src[:, :], idx_t[:, (t0 + s0) * 8:(t0 + s0 + n) * 8],
                                n * P, n * P, FIN)
                    o_t = outp.tile([P, G * FIN], BF16, tag="o")
                    for vi, vt in enumerate(vts):
                        subs = vt_subs[vt]
                        ps = psp.tile([P, FIN], F32, tag="acc")
                        half_count = [sum(1 for _, h in subs if h == hh) for hh in (0, 1)]
                        seen = [0, 0]
                        for (t, h) in subs:
                            s = t - t0
                            nc.tensor.matmul(
                                ps[h * GSZ:(h + 1) * GSZ, :],
                                sw_t[:, s * GSZ:(s + 1) * GSZ],
                                msg_t[:, s * FIN:(s + 1) * FIN],
                                start=(seen[h] == 0), stop=(seen[h] == half_count[h] - 1))
                            seen[h] += 1
                        nc.scalar.activation(o_t[:, vi * FIN:(vi + 1) * FIN], ps[:], AF.Copy)
                    nc.sync.dma_start(
                        zn[j].rearrange("(g vt p) f -> g p vt f", vt=G, p=P)[g],
                        o_t[:].rearrange("p (vt f) -> p vt f", vt=G))

        # ---- GEMM + BN stats ----
        with ExitStack() as gctx:
            zcp = gctx.enter_context(tc.tile_pool(name="zcp", bufs=2))
            big = gctx.enter_context(tc.tile_pool(name="big", bufs=1))
            psG = gctx.enter_context(tc.tile_pool(name="psG", bufs=2, space="PSUM"))
            rawT_sb = big.tile([P, 2 * V], BF16, tag="rawT")
            stats_sb = big.tile([P, 2 * NCH * 6], F32, tag="stats")
            for c in range(NCH):
                zc = zcp.tile([P, 8 * CW], BF16, tag="zc")
                for j in range(K):
                    nc.sync.dma_start_transpose(
                        zc[:].rearrange("p (k n) -> p k n", k=8)[:, 2 * j:2 * j + 2, :],
                        zn[j][c * CW:(c + 1) * CW, :])
                for oh in range(2):
                    pg = psG.tile([P, CW], F32, tag="pg")
                    for kt in range(8):
                        nc.tensor.matmul(
                            pg[:], wf_t[:, kt * FOUT + oh * P: kt * FOUT + oh * P + P],
                            zc[:, kt * CW:(kt + 1) * CW],
                            start=(kt == 0), stop=(kt == 7))
                    nc.vector.bn_stats(stats_sb[:, (oh * NCH + c) * 6:(oh * NCH + c) * 6 + 6], pg[:])
                    nc.scalar.activation(rawT_sb[:, oh * V + c * CW: oh * V + (c + 1) * CW], pg[:], AF.Copy)
            for oh in range(2):
                nc.sync.dma_start(rawT[oh], rawT_sb[:, oh * V:(oh + 1) * V])
            aggr = big.tile([P, 4], F32, tag="aggr")
            for oh in range(2):
                nc.vector.bn_aggr(aggr[:, oh * 2:oh * 2 + 2],
                                  stats_sb[:, oh * NCH * 6:(oh + 1) * NCH * 6])
            # stats out: [mean_h0, ex2_h0, mean_h1, ex2_h1]
            so = big.tile([P, 4], F32, tag="so")
            for oh in range(2):
                m = aggr[:, oh * 2:oh * 2 + 1]
                v_ = aggr[:, oh * 2 + 1:oh * 2 + 2]
                nc.vector.tensor_copy(so[:, oh * 2:oh * 2 + 1], m)
                nc.vector.tensor_tensor(out=so[:, oh * 2 + 1:oh * 2 + 2], in0=m, in1=m,
                                        op=mybir.AluOpType.mult)
                nc.vector.tensor_tensor(out=so[:, oh * 2 + 1:oh * 2 + 2],
                                        in0=so[:, oh * 2 + 1:oh * 2 + 2], in1=v_,
                                        op=mybir.AluOpType.add)
            nc.sync.dma_start(stats[:, :], so[:])
    nc.compile()
    return nc


def _build_launch_b():
    nc = bacc.Bacc("TRN2", target_bir_lowering=False, debug=False, num_devices=8)
    rawT = nc.dram_tensor("rawT", [2, P, V], BF16, kind="ExternalInput").ap()
    sc = nc.dram_tensor("sc", [P, 2], F32, kind="ExternalInput").ap()
    sh = nc.dram_tensor("sh", [P, 2], F32, kind="ExternalInput").ap()
    out = nc.dram_tensor("out", [V, FOUT], BF16, kind="ExternalOutput").ap()
    CH2 = 8           # vtiles per chunk
    NC2 = NVT // CH2  # 12 chunks
    with tile.TileContext(nc) as tc, ExitStack() as ctx:
        cpool = ctx.enter_context(tc.tile_pool(name="const", bufs=1))
        sc_t = cpool.tile([P, 2], F32, tag="sc")
        sh_t = cpool.tile([P, 2], F32, tag="sh")
        nc.sync.dma_start(sc_t[:], sc[:, :])
        nc.sync.dma_start(sh_t[:], sh[:, :])
        pool = ctx.enter_context(tc.tile_pool(name="sb", bufs=2))
        for c in range(NC2):
            nt = pool.tile([P, 2 * CH2 * P], BF16, tag="nt")
            for oh in range(2):
                nc.sync.dma_start(nt[:, oh * CH2 * P:(oh + 1) * CH2 * P],
                                  rawT[oh][:, c * CH2 * P:(c + 1) * CH2 * P])
            for oh in range(2):
                nc.scalar.activation(
                    nt[:, oh * CH2 * P:(oh + 1) * CH2 * P],
                    nt[:, oh * CH2 * P:(oh + 1) * CH2 * P],
                    AF.Relu, bias=sh_t[:, oh:oh + 1], scale=sc_t[:, oh:oh + 1])
            ot = pool.tile([P, CH2 * FOUT], BF16, tag="ot")
            for oh in range(2):
                nc.sync.dma_start_transpose(
                    ot[:].rearrange("p (t f) -> p t f", t=CH2)[:, :, oh * P:(oh + 1) * P],
                    nt[:, oh * CH2 * P:(oh + 1) * CH2 * P])
            nc.sync.dma_start(
                out.rearrange("(c vt p) f -> c p vt f", vt=CH2, p=P)[c],
                ot[:].rearrange("p (vt f) -> p vt f", vt=CH2))
    nc.compile()
    return nc


def kernel(x, edge_weight, weight, bias, gamma, beta, edge_src, edge_dst):
    x = np.asarray(x, np.float32)
    edge_weight = np.asarray(edge_weight, np.float32)
    weight = np.asarray(weight, np.float32)
    gamma = np.asarray(gamma, np.float32)
    beta = np.asarray(beta, np.float32)
    edge_src = np.asarray(edge_src, np.int32)
    edge_dst = np.asarray(edge_dst, np.int32)

    idx_np, sw, vt_subs, ST = _build_schedule(edge_src, edge_dst, edge_weight)
    key = ("A", ST, tuple(len(s) for s in vt_subs))
    if key not in _cache:
        _cache[key] = _build_launch_a(ST, vt_subs)
    ncA = _cache[key]
    if "B" not in _cache:
        _cache["B"] = _build_launch_b()
    ncB = _cache["B"]

    wf = _fold_weights(weight)
    swt = np.ascontiguousarray(sw.transpose(1, 0, 2)).reshape(P, ST * GSZ).astype(BF16_NP)
    idx16 = _wrap_idx(idx_np, ST)                      # [128, ST*8] int16
    in_maps = []
    flat_idx = idx_np.reshape(-1)
    for b in range(B):
        xb = x[b].astype(BF16_NP)                      # [V, FIN]
        msg0 = np.ascontiguousarray(
            xb[flat_idx].reshape(ST, P, FIN).transpose(1, 0, 2))
        in_maps.append({
            "xb": xb, "msg0": msg0,
            "idx": idx16, "swt": swt, "wf": wf,
        })
    resA = run_bass_kernel_spmd(ncA, in_maps, core_ids=list(range(B)))

    # host: combine BN stats across cores (equal counts -> simple average)
    st = np.stack([resA.results[b]["stats"] for b in range(B)])   # [B, 128, 4]
    mean = st[:, :, [0, 2]].mean(0)                               # [128, 2]
    ex2 = st[:, :, [1, 3]].mean(0)
    var = ex2 - mean * mean
    g2 = gamma.reshape(2, P).T                                    # [128, 2]
    b2 = beta.reshape(2, P).T
    scale = (g2 / np.sqrt(var + EPS)).astype(np.float32)
    shift = (b2 - mean * scale).astype(np.float32)

    in_maps_b = [{"rawT": resA.results[b]["rawT"], "sc": scale, "sh": shift}
                 for b in range(B)]
    resB = run_bass_kernel_spmd(ncB, in_maps_b, core_ids=list(range(B)))
    out = np.stack([np.asarray(resB.results[b]["out"]).astype(np.float32)
                    for b in range(B)])
    # bias cancels inside training-mode BN (shifts the mean only); gamma/beta applied above
    return out


# revision 16
# speedup vs baseline: 3.2291x; 1.0257x over previous
"""Trainium2 Bass kernel for nn_ConvBlock (Chebyshev graph conv + BatchNorm + ReLU).

Sharding: data-parallel over batch (B=8 -> 1 sample per NeuronCore).
Per core: Chebyshev recursion in the power basis (z_j = L^j x) with edge-row
gathers done by batched SWDGE dma_gather (bf16, one call per 8 node-vtiles)
and scatter via one-hot matmuls (edge weights folded into host-built scatter
blocks). The recursion also emits z_j pre-transposed (feature-major zT) so the
K-stacked GEMM consumes it directly with no on-device transposes. BN stats are
computed on device, combined across cores on host between two launches;
launch B applies scale/shift + ReLU and transposes back to node-major.
All matmul/DMA traffic is bf16 (PSUM accumulation fp32); host does fp32<->bf16.
"""
import os, sys
sys.path.insert(0, '/opt/trn_rl_repo')
import numpy as np
import ml_dtypes
from contextlib import ExitStack

import concourse.bass as bass
import concourse.tile as tile
from concourse import bacc, mybir
from concourse.bass_utils import run_bass_kernel_spmd
from concourse.masks import make_identity
from concourse.library_config import mlp

B, V, E = 8, 12288, 98304
FIN, FOUT, K = 256, 256, 4
EPS = 1e-5
P = 128
GSZ = 64            # dst-group node window (S_w block width)
NVT = V // P        # 96 vtiles (group pairs)
G = 4               # vtiles per gather/compute group in the recursion
NGRP = NVT // G     # 24 groups
NCH = 24            # GEMM chunks of 512 nodes
CW = V // NCH       # 512

F32 = mybir.dt.float32
BF16 = mybir.dt.bfloat16
I16 = mybir.dt.int16
AF = mybir.ActivationFunctionType
BF16_NP = ml_dtypes.bfloat16

_cache = {}


def _build_schedule(edge_src, edge_dst, edge_weight):
    """Group edges by 64-node dst windows, pad each group to multiples of 128."""
    g_of_e = edge_dst // GSZ
    order = np.argsort(g_of_e, kind='stable')
    NG = V // GSZ
    counts = np.bincount(g_of_e, minlength=NG)
    sub_of_g = np.maximum(1, (counts + P - 1) // P)   # subtiles per group
    ST = int(sub_of_g.sum())
    idx_np = np.zeros((ST, P), np.int32)              # src per (subtile, lane)
    sw = np.zeros((ST, P, GSZ), np.float32)           # scatter blocks
    vt_subs = [[] for _ in range(NVT)]                # subtile ids per vtile half
    t = 0
    pos = 0
    for g in range(NG):
        eg = order[pos:pos + counts[g]]
        pos += counts[g]
        for s in range(sub_of_g[g]):
            part = eg[s * P:(s + 1) * P]
            n = len(part)
            idx_np[t, :n] = edge_src[part]
            sw[t, np.arange(n), edge_dst[part] - g * GSZ] = edge_weight[part]
            vt_subs[g // 2].append((t, g % 2))
            t += 1
    assert t == ST
    return idx_np, sw, vt_subs, ST


def _fold_weights(weight):
    # out = sum_k T_k(L) x W_k ; T0=I, T1=L, T2=2L^2-1, T3=4L^3-3L
    # power basis z_j = L^j x :  out = sum_j z_j Wf_j
    W = weight
    Wf = np.stack([W[0] - W[2], W[1] - 3.0 * W[3], 2.0 * W[2], 4.0 * W[3]])
    # [(j,i), o] -> tiles [8, 128, 256]
    return Wf.reshape(K * FIN, FOUT).reshape(8, P, FOUT).astype(BF16_NP)


def _wrap_idx(idx_np, ST):
    """dma_gather index layout: idx i at [i%16, i//16], replicated to 128 rows."""
    # idx_np: [ST, 128] int; per subtile t lane l -> position t*128+l
    blk = idx_np.astype(np.int16).reshape(ST, 8, 16).transpose(2, 0, 1).reshape(16, ST * 8)
    return np.ascontiguousarray(np.tile(blk, (8, 1)))  # [128, ST*8]


def _build_launch_a(ST, vt_subs):
    nc = bacc.Bacc("TRN2", target_bir_lowering=False, debug=False, num_devices=8)
    xb = nc.dram_tensor("xb", [V, FIN], BF16, kind="ExternalInput").ap()
    msg0 = nc.dram_tensor("msg0", [P, ST, FIN], BF16, kind="ExternalInput").ap()
    idx = nc.dram_tensor("idx", [P, ST * 8], I16, kind="ExternalInput").ap()
    swt = nc.dram_tensor("swt", [P, ST * GSZ], BF16, kind="ExternalInput").ap()
    wf = nc.dram_tensor("wf", [8, P, FOUT], BF16, kind="ExternalInput").ap()
    rawT = nc.dram_tensor("rawT", [2, P, V], BF16, kind="ExternalOutput").ap()
    stats = nc.dram_tensor("stats", [P, 4], F32, kind="ExternalOutput").ap()
    zn = [xb] + [nc.dram_tensor(f"z{j}", [V, FIN], BF16).ap() for j in (1, 2, 3)]

    # group boundaries: (t0, nst, [(vt, subs)...]) per G-vtile group
    groups = []
    for g in range(NGRP):
        vts = list(range(g * G, (g + 1) * G))
        subs_all = [s for vt in vts for s in vt_subs[vt]]
        groups.append((subs_all[0][0], len(subs_all), vts))

    with tile.TileContext(nc) as tc, ExitStack() as ctx:
        cpool = ctx.enter_context(tc.tile_pool(name="const", bufs=1))
        nc.gpsimd.load_library(mlp)
        wf_t = cpool.tile([P, 8 * FOUT], BF16, tag="wf")
        nc.sync.dma_start(wf_t[:].rearrange("p (k o) -> p k o", k=8), wf.transpose([1, 0, 2]))

        # ---- Chebyshev (power-basis) recursion: z_j = L z_{j-1} ----
        with ExitStack() as rctx:
            rcpool = rctx.enter_context(tc.tile_pool(name="rconst", bufs=1))
            idx_t = rcpool.tile([P, ST * 8], I16, tag="idx")
            nc.sync.dma_start(idx_t[:], idx[:, :])
            sw_all = rcpool.tile([P, ST * GSZ], BF16, tag="swall")
            nc.sync.dma_start(sw_all[:], swt[:, :])
            msgp = rctx.enter_context(tc.tile_pool(name="msgp", bufs=2))
            outp = rctx.enter_context(tc.tile_pool(name="outp", bufs=2))
            psp = rctx.enter_context(tc.tile_pool(name="psp", bufs=4, space="PSUM"))
            for j in (1, 2, 3):
                src = zn[j - 1]
                for (t0, nst, vts) in groups:
                    g = vts[0] // G
                    msg_t = msgp.tile([P, nst * FIN], BF16, tag="msg")
                    if j == 1:
                        nc.sync.dma_start(
                            msg_t[:].rearrange("p (t f) -> p t f", t=nst),
                            msg0[:, t0:t0 + nst, :])
                    else:
                        # SWDGE ucode caps one gather call at 1024 descriptors
                        for s0 in range(0, nst, 8):
                            n = min(8, nst - s0)
                            nc.gpsimd.dma_gather(
                                msg_t[:].rearrange("p (t f) -> p t f", t=nst)[:, s0:s0 + n, :],
                                src[:, :], idx_t[:, (t0 + s0) * 8:(t0 + s0 + n) * 8],
                                n * P, n * P, FIN)
                    o_t = outp.tile([P, G * FIN], BF16, tag="o")
                    for vi, vt in enumerate(vts):
                        subs = vt_subs[vt]
                        ps = psp.tile([P, FIN], F32, tag="acc")
                        half_count = [sum(1 for _, h in subs if h == hh) for hh in (0, 1)]
                        seen = [0, 0]
                        for (t, h) in subs:
                            s = t - t0
                            nc.tensor.matmul(
                                ps[h * GSZ:(h + 1) * GSZ, :],
                                sw_all[:, t * GSZ:(t + 1) * GSZ],
                                msg_t[:, s * FIN:(s + 1) * FIN],
                                start=(seen[h] == 0), stop=(seen[h] == half_count[h] - 1))
                            seen[h] += 1
                        nc.scalar.activation(o_t[:, vi * FIN:(vi + 1) * FIN], ps[:], AF.Copy)
                    nc.sync.dma_start(
                        zn[j].rearrange("(g vt p) f -> g p vt f", vt=G, p=P)[g],
                        o_t[:].rearrange("p (vt f) -> p vt f", vt=G))

        # ---- GEMM + BN stats ----
        with ExitStack() as gctx:
            zcp = gctx.enter_context(tc.tile_pool(name="zcp", bufs=2))
            big = gctx.enter_context(tc.tile_pool(name="big", bufs=1))
            psG = gctx.enter_context(tc.tile_pool(name="psG", bufs=2, space="PSUM"))
            rawT_sb = big.tile([P, 2 * V], BF16, tag="rawT")
            stats_sb = big.tile([P, 2 * NCH * 6], F32, tag="stats")
            for c in range(NCH):
                zc = zcp.tile([P, 8 * CW], BF16, tag="zc")
                for j in range(K):
                    nc.sync.dma_start_transpose(
                        zc[:].rearrange("p (k n) -> p k n", k=8)[:, 2 * j:2 * j + 2, :],
                        zn[j][c * CW:(c + 1) * CW, :])
                for oh in range(2):
                    pg = psG.tile([P, CW], F32, tag="pg")
                    for kt in range(8):
                        nc.tensor.matmul(
                            pg[:], wf_t[:, kt * FOUT + oh * P: kt * FOUT + oh * P + P],
                            zc[:, kt * CW:(kt + 1) * CW],
                            start=(kt == 0), stop=(kt == 7))
                    nc.vector.bn_stats(stats_sb[:, (oh * NCH + c) * 6:(oh * NCH + c) * 6 + 6], pg[:])
                    nc.scalar.activation(rawT_sb[:, oh * V + c * CW: oh * V + (c + 1) * CW], pg[:], AF.Copy)
            for oh in range(2):
                nc.sync.dma_start(rawT[oh], rawT_sb[:, oh * V:(oh + 1) * V])
            aggr = big.tile([P, 4], F32, tag="aggr")
            for oh in range(2):
                nc.vector.bn_aggr(aggr[:, oh * 2:oh * 2 + 2],
                                  stats_sb[:, oh * NCH * 6:(oh + 1) * NCH * 6])
            # stats out: [mean_h0, ex2_h0, mean_h1, ex2_h1]
            so = big.tile([P, 4], F32, tag="so")
            for oh in range(2):
                m = aggr[:, oh * 2:oh * 2 + 1]
                v_ = aggr[:, oh * 2 + 1:oh * 2 + 2]
                nc.vector.tensor_copy(so[:, oh * 2:oh * 2 + 1], m)
                nc.vector.tensor_tensor(out=so[:, oh * 2 + 1:oh * 2 + 2], in0=m, in1=m,
                                        op=mybir.AluOpType.mult)
                nc.vector.tensor_tensor(out=so[:, oh * 2 + 1:oh * 2 + 2],
                                        in0=so[:, oh * 2 + 1:oh * 2 + 2], in1=v_,
                                        op=mybir.AluOpType.add)
            nc.sync.dma_start(stats[:, :], so[:])
    nc.compile()
    return nc


def _build_launch_b():
    nc = bacc.Bacc("TRN2", target_bir_lowering=False, debug=False, num_devices=8)
    rawT = nc.dram_tensor("rawT", [2, P, V], BF16, kind="ExternalInput").ap()
    sc = nc.dram_tensor("sc", [P, 2], F32, kind="ExternalInput").ap()
    sh = nc.dram_tensor("sh", [P, 2], F32, kind="ExternalInput").ap()
    out = nc.dram_tensor("out", [V, FOUT], BF16, kind="ExternalOutput").ap()
    CH2 = 8           # vtiles per chunk
    NC2 = NVT // CH2  # 12 chunks
    with tile.TileContext(nc) as tc, ExitStack() as ctx:
        cpool = ctx.enter_context(tc.tile_pool(name="const", bufs=1))
        sc_t = cpool.tile([P, 2], F32, tag="sc")
        sh_t = cpool.tile([P, 2], F32, tag="sh")
        nc.sync.dma_start(sc_t[:], sc[:, :])
        nc.sync.dma_start(sh_t[:], sh[:, :])
        pool = ctx.enter_context(tc.tile_pool(name="sb", bufs=2))
        for c in range(NC2):
            nt = pool.tile([P, 2 * CH2 * P], BF16, tag="nt")
            for oh in range(2):
                nc.sync.dma_start(nt[:, oh * CH2 * P:(oh + 1) * CH2 * P],
                                  rawT[oh][:, c * CH2 * P:(c + 1) * CH2 * P])
            for oh in range(2):
                nc.scalar.activation(
                    nt[:, oh * CH2 * P:(oh + 1) * CH2 * P],
                    nt[:, oh * CH2 * P:(oh + 1) * CH2 * P],
                    AF.Relu, bias=sh_t[:, oh:oh + 1], scale=sc_t[:, oh:oh + 1])
            ot = pool.tile([P, CH2 * FOUT], BF16, tag="ot")
            for oh in range(2):
                nc.sync.dma_start_transpose(
                    ot[:].rearrange("p (t f) -> p t f", t=CH2)[:, :, oh * P:(oh + 1) * P],
                    nt[:, oh * CH2 * P:(oh + 1) * CH2 * P])
            nc.sync.dma_start(
                out.rearrange("(c vt p) f -> c p vt f", vt=CH2, p=P)[c],
                ot[:].rearrange("p (vt f) -> p vt f", vt=CH2))
    nc.compile()
    return nc


def kernel(x, edge_weight, weight, bias, gamma, beta, edge_src, edge_dst):
    x = np.asarray(x, np.float32)
    edge_weight = np.asarray(edge_weight, np.float32)
    weight = np.asarray(weight, np.float32)
    gamma = np.asarray(gamma, np.float32)
    beta = np.asarray(beta, np.float32)
    edge_src = np.asarray(edge_src, np.int32)
    edge_dst = np.asarray(edge_dst, np.int32)

    idx_np, sw, vt_subs, ST = _build_schedule(edge_src, edge_dst, edge_weight)
    key = ("A", ST, tuple(len(s) for s in vt_subs))
    if key not in _cache:
        _cache[key] = _build_launch_a(ST, vt_subs)
    ncA = _cache[key]
    if "B" not in _cache:
        _cache["B"] = _build_launch_b()
    ncB = _cache["B"]

    wf = _fold_weights(weight)
    swt = np.ascontiguousarray(sw.transpose(1, 0, 2)).reshape(P, ST * GSZ).astype(BF16_NP)
    idx16 = _wrap_idx(idx_np, ST)                      # [128, ST*8] int16
    in_maps = []
    flat_idx = idx_np.reshape(-1)
    for b in range(B):
        xb = x[b].astype(BF16_NP)                      # [V, FIN]
        msg0 = np.ascontiguousarray(
            xb[flat_idx].reshape(ST, P, FIN).transpose(1, 0, 2))
        in_maps.append({
            "xb": xb, "msg0": msg0,
            "idx": idx16, "swt": swt, "wf": wf,
        })
    resA = run_bass_kernel_spmd(ncA, in_maps, core_ids=list(range(B)))

    # host: combine BN stats across cores (equal counts -> simple average)
    st = np.stack([resA.results[b]["stats"] for b in range(B)])   # [B, 128, 4]
    mean = st[:, :, [0, 2]].mean(0)                               # [128, 2]
    ex2 = st[:, :, [1, 3]].mean(0)
    var = ex2 - mean * mean
    g2 = gamma.reshape(2, P).T                                    # [128, 2]
    b2 = beta.reshape(2, P).T
    scale = (g2 / np.sqrt(var + EPS)).astype(np.float32)
    shift = (b2 - mean * scale).astype(np.float32)

    in_maps_b = [{"rawT": resA.results[b]["rawT"], "sc": scale, "sh": shift}
                 for b in range(B)]
    resB = run_bass_kernel_spmd(ncB, in_maps_b, core_ids=list(range(B)))
    out = np.stack([np.asarray(resB.results[b]["out"]).astype(np.float32)
                    for b in range(B)])
    # bias cancels inside training-mode BN (shifts the mean only); gamma/beta applied above
    return out
